# revision 70
# baseline (speedup 1.0000x reference)
"""Trainium2 Bass kernel for nn_BlockMask (multi-modal masked transformer).

Strategy: data-parallel over batch (B=32 -> 4 samples on each of 8 cores).
Activations are kept feature-major [C-on-partitions, tokens-free] so every
GEMM uses the weights in their natural [K, N] layout as lhsT with zero
transposes in the main path.  LayerNorm statistics are computed with
ones-vector matmuls on the PE; per-token scale/mean are broadcast across
partitions with PE rank-1 matmuls.  The masked softmax runs in
[k-on-partitions, q-free] layout so the key mask is a per-partition bias on
the Exp activation and the denominator comes from an extra ones column
appended to V.  Big GEMMs run in float32r (full-rate, fp32 storage);
attention internals (q/k/v/e) run in bf16.  The MLP is pass-interleaved:
fc1 produces one 768-wide hidden block, fc2 immediately consumes it and
accumulates into the SBUF residual staging tile, so only 12 weight tiles
of [128,768] are ever live.
"""

import sys

if "/opt/trn_rl_repo" not in sys.path:
    sys.path.insert(0, "/opt/trn_rl_repo")

import numpy as np

B = 32
NCORES = 8
S = B // NCORES          # samples per core = 4
N = 129
C = 768
CO = C // 128            # 6
H = 12
HD = 64
HID = 4 * C
T = S * N                # 516
N4 = 3 * N               # 387
T4 = S * N4              # 1548
EPS = 1e-5
NEGB = -30000.0
SCL = HD ** -0.5

_CACHE = {}


def _build(has_b1, has_b2, has_bf):
    import concourse.bass as bass
    import concourse.mybir as mybir
    from concourse import bacc
    from concourse.tile import TileContext
    from concourse.masks import make_identity
    from contextlib import ExitStack

    dt = mybir.dt
    f32 = dt.float32
    f32r = dt.float32r
    bf16 = dt.bfloat16
    AF = mybir.ActivationFunctionType
    OP = mybir.AluOpType

    nc = bacc.Bacc("TRN2", target_bir_lowering=False, debug=False,
                   num_devices=NCORES)

    xin = {}
    for nm in ("RGB", "NIR", "TIR"):
        xin[nm] = nc.dram_tensor(nm, [S, N, C], f32, kind="ExternalInput")
    mask_d = nc.dram_tensor("mask", [S, N - 1, 1], f32, kind="ExternalInput")
    lng1 = nc.dram_tensor("ln_g1", [4, C], f32, kind="ExternalInput")
    lnb1 = nc.dram_tensor("ln_b1", [4, C], f32, kind="ExternalInput")
    lng2 = nc.dram_tensor("ln_g2", [4, C], f32, kind="ExternalInput")
    lnb2 = nc.dram_tensor("ln_b2", [4, C], f32, kind="ExternalInput")
    wqkv_d = nc.dram_tensor("w_qkv", [4, C, 3 * C], f32, kind="ExternalInput")
    wproj_d = nc.dram_tensor("w_proj", [4, C, C], f32, kind="ExternalInput")
    wfc1_d = nc.dram_tensor("w_fc1", [4, C, HID], f32, kind="ExternalInput")
    wfc2_d = nc.dram_tensor("w_fc2", [4, HID, C], f32, kind="ExternalInput")
    outg_d = nc.dram_tensor("out_g", [C], f32, kind="ExternalInput")
    outb_d = nc.dram_tensor("out_b", [C], f32, kind="ExternalInput")
    y_d = nc.dram_tensor("y", [S, N4, C], f32, kind="ExternalOutput")

    ctx = ExitStack()
    with TileContext(nc) as tc, ctx:
        wpool = ctx.enter_context(tc.tile_pool(name="wpool", bufs=12))
        xmpool = ctx.enter_context(tc.tile_pool(name="xmpool", bufs=2))
        xspool = ctx.enter_context(tc.tile_pool(name="xspool", bufs=3))
        qkpool = ctx.enter_context(tc.tile_pool(name="qkpool", bufs=2))
        vpool = ctx.enter_context(tc.tile_pool(name="vpool", bufs=2))
        epool = ctx.enter_context(tc.tile_pool(name="epool", bufs=4))
        atpool = ctx.enter_context(tc.tile_pool(name="atpool", bufs=2))
        hppool = ctx.enter_context(tc.tile_pool(name="hppool", bufs=2))
        rwpool = ctx.enter_context(tc.tile_pool(name="rwpool", bufs=4))
        gbpool = ctx.enter_context(tc.tile_pool(name="gbpool", bufs=4))
        cpool = ctx.enter_context(tc.tile_pool(name="cpool", bufs=1))
        mmps = ctx.enter_context(tc.tile_pool(name="mmps", bufs=6, space="PSUM"))
        stps = ctx.enter_context(tc.tile_pool(name="stps", bufs=2, space="PSUM"))
        dram = ctx.enter_context(tc.tile_pool(name="dram", bufs=1, space="DRAM"))

        def mm_ps(p, f, name):
            return mmps.tile([p, f], f32, tag="mm", name=name)

        def zero_fill_r(ap):
            """zero an fp32r-consumed region via DVE (memset can't write
            fp32r-typed and non-rounded writers fail BIR verification)."""
            zc = nc.const_aps.aps[(f32, 0.0)]
            src = zc[0:ap.shape[0]]
            while len(src.shape) < len(ap.shape):
                src = src[..., None]
            nc.vector.tensor_copy(ap.bitcast(f32r), src.to_broadcast(ap.shape))

        # ---------------- constants ----------------
        ident = cpool.tile([128, 128], f32, tag="ident", name="ident")
        make_identity(nc, ident)
        ones_f = cpool.tile([128, 1], f32, tag="onesf", name="ones_f")
        nc.gpsimd.memset(ones_f, 1.0)
        # memset can't write float32r; route through a DVE copy
        ones_col = cpool.tile([128, 1], f32r, tag="onesc", name="ones_col")
        nc.vector.tensor_copy(ones_col, ones_f)
        ones_row = cpool.tile([1, 128], f32, tag="onesr", name="ones_row")
        nc.gpsimd.memset(ones_row, 1.0)
        ones_row_r = cpool.tile([1, 128], f32r, tag="onesrr", name="ones_row_r")
        nc.vector.tensor_copy(ones_row_r, ones_row)
        eps_col = cpool.tile([128, 1], f32, tag="epsc", name="eps_col")
        nc.gpsimd.memset(eps_col, EPS)
        # head-pair replication rows: partition 0 -> cols 0:64,
        # partition 64 -> cols 64:128 (engine SBUF operands must start at
        # partition 0/32/64/96, and matmul lhsT/rhs bases must match)
        e2f = cpool.tile([128, 128], f32, tag="e2f", name="e2f")
        nc.gpsimd.memset(e2f[0:1, :], 0.0)
        nc.gpsimd.memset(e2f[64:65, :], 0.0)
        nc.gpsimd.memset(e2f[0:1, 0:HD], 1.0)
        nc.gpsimd.memset(e2f[64:65, HD:128], 1.0)
        e2 = cpool.tile([128, 128], f32r, tag="e2", name="e2")
        nc.vector.tensor_copy(e2[0:1], e2f[0:1])
        nc.vector.tensor_copy(e2[64:65], e2f[64:65])

        m_row = cpool.tile([1, T], f32, tag="mrow", name="m_row")
        nc.gpsimd.memset(m_row, 1.0)
        mr_v = m_row.rearrange("one (s n) -> one s n", s=S)
        nc.sync.dma_start(mr_v[:, :, 1:N],
                          mask_d.ap().rearrange("s n one -> one s n"))
        # padded by 4 so 388-wide (even-padded) slices at the last sample stay
        # in bounds; pad values are 1.0 and only ever touch dead lanes
        m3_row = cpool.tile([1, T4 + 4], f32, tag="m3row", name="m3_row")
        nc.gpsimd.memset(m3_row, 1.0)
        nc.vector.tensor_copy(
            m3_row[0:1, 0:T4].rearrange("one (s g n) -> one s g n", s=S, g=3),
            mr_v[:, :, None, :].to_broadcast((1, S, 3, N)))

        def bcast_row(row_ap, F, name):
            t_sb = cpool.tile([128, F], f32, tag=name, name=name)
            o = 0
            while o < F:
                w = min(512, F - o)
                ps = mm_ps(128, w, f"bc_{name}_{o}")
                nc.tensor.matmul(ps, lhsT=ones_row, rhs=row_ap[0:1, o:o + w],
                                 start=True, stop=True)
                nc.scalar.copy(t_sb[:, o:o + w], ps)
                o += w
            return t_sb

        # only consumed by the LN bias terms (bias==0 for this problem)
        m_bc = bcast_row(m_row, T, "mbc") if (has_b1 or has_b2) else None
        m3_bc = (bcast_row(m3_row, T4 + 4, "m3bc")
                 if (has_b1 or has_b2 or has_bf) else None)

        def build_negm(row_ap, kcs, nseq, name):
            t = cpool.tile([128, S, len(kcs)], f32, tag=name, name=name)
            nc.gpsimd.memset(t, 0.0)
            for s in range(S):
                for j, (t0, kcp) in enumerate(kcs):
                    ps = mmps.tile([kcp, 1], f32, tag="mm",
                                   name=f"{name}_{s}_{j}")
                    nc.tensor.matmul(
                        ps,
                        lhsT=row_ap[0:1, s * nseq + t0:s * nseq + t0 + kcp],
                        rhs=ident[0:1, 0:1], start=True, stop=True)
                    nc.scalar.copy(t[0:kcp, s:s + 1, j], ps)
            nc.vector.tensor_scalar(t, t, -NEGB, NEGB, OP.mult, OP.add)
            return t

        KCS3 = [(0, 128), (128, 1)]
        KCS4 = [(0, 128), (128, 128), (256, 128), (384, 3)]
        negm3 = build_negm(m_row, KCS3, N, "negm3")
        negm4 = build_negm(m3_row, KCS4, N4, "negm4")

        xb_dram = [dram.tile([CO, 128, T], f32, name=f"xb{b}")
                   for b in range(3)]
        xd4 = dram.tile([CO, 128, T4], f32, name="xd4")
        xd4s = xd4.rearrange("co ci (s t) -> co ci s t", s=S)

        def ln_cols(dten, idx, name):
            t = gbpool.tile([128, CO], f32, tag="gb", name=name)
            src = dten.ap()[idx] if idx is not None else dten.ap()
            nc.sync.dma_start(t, src.rearrange("(co ci) -> ci co", ci=128))
            return t

        def wtile(src_ap, name):
            """load a [128, 768] weight tile (typed fp32r for the PE)."""
            t = wpool.tile([128, C], f32r, tag="w", name=name)
            nc.sync.dma_start(t, src_ap.bitcast(f32r))
            return t

        # ---------------- generic layernorm ----------------
        def emit_ln(xsrc_ap, mrow_ap, mbc_ap, g_sb, b_sb, F, nm, Fv=None):
            """F = (even-padded) tile width, Fv = valid source width."""
            Fv = F if Fv is None else Fv
            xs = xspool.tile([128, CO, F], f32, tag="xst", name=f"xs_{nm}")
            nc.sync.dma_start(xs[:, :, 0:Fv].bitcast(f32r),
                              xsrc_ap.bitcast(f32r))
            if Fv < F:
                zero_fill_r(xs[:, :, Fv:F])
            st_s = stps.tile([1, F], f32, tag="st", name=f"sts_{nm}")
            st_q = stps.tile([1, F], f32, tag="st", name=f"stq_{nm}")
            oc = ones_col
            for co in range(CO):
                nc.tensor.matmul(st_s, lhsT=oc, rhs=xs[:, co, :].bitcast(f32r),
                                 start=(co == 0), stop=(co == CO - 1))
            for co in range(CO):
                x2 = xmpool.tile([128, F], f32, tag="xmsq",
                                 name=f"x2_{nm}_{co}")
                # square on the (otherwise idle) gpsimd engine
                nc.gpsimd.tensor_tensor(x2.bitcast(f32r), xs[:, co, :],
                                        xs[:, co, :], OP.mult)
                nc.tensor.matmul(st_q, lhsT=oc, rhs=x2.bitcast(f32r),
                                 start=(co == 0), stop=(co == CO - 1))
            mu = rwpool.tile([1, F], f32, tag="lnrow", name=f"mu_{nm}")
            nc.vector.tensor_scalar_mul(mu.bitcast(f32r), st_s, 1.0 / C)
            var = rwpool.tile([1, F], f32, tag="lnrow", name=f"var_{nm}")
            # var = sumsq/C - mu^2:  (st_q * 1/C) then subtract mu*mu
            nc.vector.tensor_scalar_mul(var, st_q, 1.0 / C)
            mu2 = rwpool.tile([1, F], f32, tag="lnrow", name=f"mu2_{nm}")
            nc.vector.tensor_tensor(mu2, mu, mu, OP.mult)
            nc.vector.tensor_tensor(var, var, mu2, OP.subtract)
            std = rwpool.tile([1, F], f32, tag="lnrow", name=f"sd_{nm}")
            nc.scalar.activation(std, var, AF.Sqrt, bias=eps_col[0:1])
            rstd = rwpool.tile([1, F], f32, tag="lnrow", name=f"rs_{nm}")
            nc.vector.reciprocal(rstd, std)
            arow = rwpool.tile([1, F], f32, tag="lnrow", name=f"a_{nm}")
            nc.vector.tensor_tensor(arow.bitcast(f32r), rstd, mrow_ap, OP.mult)
            bc_mu = mm_ps(128, F, f"bcmu_{nm}")
            nc.tensor.matmul(bc_mu, lhsT=ones_row_r, rhs=mu.bitcast(f32r),
                             start=True, stop=True)
            bc_a = mm_ps(128, F, f"bca_{nm}")
            nc.tensor.matmul(bc_a, lhsT=ones_row_r, rhs=arow.bitcast(f32r),
                             start=True, stop=True)
            xm = xmpool.tile([128, CO, F], f32, tag="xm", name=f"xm_{nm}")
            for co in range(CO):
                tmp = xmpool.tile([128, F], f32, tag="xmsq",
                                  name=f"tmp_{nm}_{co}")
                nc.vector.tensor_tensor(tmp, xs[:, co, :], bc_mu, OP.subtract)
                nc.vector.scalar_tensor_tensor(
                    xm[:, co, :].bitcast(f32r), tmp, g_sb[:, co:co + 1], bc_a,
                    OP.mult, OP.mult)
            if b_sb is not None:
                for co in range(CO):
                    nc.vector.scalar_tensor_tensor(
                        xm[:, co, :].bitcast(f32r), mbc_ap,
                        b_sb[:, co:co + 1], xm[:, co, :], OP.mult, OP.add)
            return xm

        # ---------------- branch ----------------
        def emit_branch(b):
            b4 = (b == 3)
            nseq = N4 if b4 else N
            kcs = KCS4 if b4 else KCS3
            negm = negm4 if b4 else negm3
            mrow = m3_row if b4 else m_row
            mbc = m3_bc if b4 else m_bc
            TT = T4 if b4 else T
            FQv = N4 if b4 else 2 * N      # valid qkv chunk: 387 / 258
            FQ = 388 if b4 else 2 * N      # even-padded tile width for fp32r
            nsl = 1 if b4 else 2           # samples per qkv chunk

            def x_ap(t0, F):
                src = xd4 if b4 else xb_dram[b]
                return src[:, :, t0:t0 + F].rearrange("co ci t -> ci co t")

            g1 = ln_cols(lng1, b, f"g1_{b}")
            b1 = ln_cols(lnb1, b, f"b1_{b}") if has_b1 else None
            g2 = ln_cols(lng2, b, f"g2_{b}")
            b2 = ln_cols(lnb2, b, f"b2_{b}") if has_b2 else None
            wq = wqkv_d.ap()[b]

            # ---- qkv + attention, per superblock of 2 samples ----
            atts = {}
            nsb = 1 if not b4 else 2
            for sb in range(nsb):
                iis = [sb * 2 + i for i in range(2)] if b4 else [0, 1]
                xms, qts, kts, vts = {}, {}, {}, {}
                for i in iis:
                    t0 = i * FQv
                    xms[i] = emit_ln(
                        x_ap(t0, FQv), mrow[0:1, t0:t0 + FQ],
                        mbc[:, t0:t0 + FQ] if b1 is not None else None,
                        g1, b1, FQ, f"ln1_{b}_{i}", Fv=FQv)
                # q pass (cols 0:768) then k pass (768:1536)
                for pi, pool_tag in ((0, "q"), (1, "k")):
                    wts = [wtile(wq[c * 128:(c + 1) * 128,
                                    pi * C:(pi + 1) * C], f"wq{pi}_{b}_{sb}_{c}")
                           for c in range(CO)]
                    for i in iis:
                        qt = qkpool.tile([128, CO, FQ], bf16, tag=pool_tag,
                                         name=f"{pool_tag}_{b}_{i}")
                        (qts if pi == 0 else kts)[i] = qt
                        for o in range(CO):
                            ps = mm_ps(128, FQ, f"qk_{b}_{i}_{pi}_{o}")
                            for c in range(CO):
                                nc.tensor.matmul(
                                    ps,
                                    lhsT=wts[c][:, o * 128:(o + 1) * 128]
                                        .bitcast(f32r),
                                    rhs=xms[i][:, c, :].bitcast(f32r),
                                    start=(c == 0), stop=(c == CO - 1))
                            nc.scalar.copy(qt[:, o, :], ps)
                # v pass (cols 1536:2304), token-major with ones column
                wts = [wtile(wq[c * 128:(c + 1) * 128, 2 * C:3 * C],
                             f"wv_{b}_{sb}_{c}") for c in range(CO)]
                for i in iis:
                    vp = vpool.tile([128, nsl, len(kcs), H, HD + 1], bf16,
                                    tag="vp", name=f"vp_{b}_{i}")
                    nc.gpsimd.memset(vp[:, :, :, :, HD:HD + 1], 1.0)
                    vts[i] = vp
                    for sl in range(nsl):
                        for j, (t0, kcp0) in enumerate(kcs):
                            # pad odd stationary widths to even (extra row of
                            # v is garbage and never read by att@v)
                            kcp = kcp0 + (kcp0 % 2) if b4 else kcp0
                            tok = sl * N + t0
                            for vh in range(2):
                                ps = mm_ps(kcp, 384, f"v_{b}_{i}_{sl}_{j}_{vh}")
                                for c in range(CO):
                                    nc.tensor.matmul(
                                        ps,
                                        lhsT=xms[i][:, c, tok:tok + kcp]
                                            .bitcast(f32r),
                                        rhs=wts[c][:, vh * 384:(vh + 1) * 384]
                                            .bitcast(f32r),
                                        start=(c == 0), stop=(c == CO - 1))
                                nc.vector.tensor_copy(
                                    vp[0:kcp, sl, j, vh * 6:(vh + 1) * 6, 0:HD],
                                    ps.rearrange("p (h d) -> p h d", d=HD))

                # ---- attention (per sample of this superblock) ----
                for i in iis:
                    at = atpool.tile([128, CO, FQ], f32, tag="at",
                                     name=f"at_{b}_{i}")
                    if FQ > FQv:
                        zero_fill_r(at[:, :, FQv:FQ])
                    atts[i] = at
                samples = iis if b4 else [0, 1, 2, 3]
                dalls = {}
                for s in samples:
                    i = s if b4 else s // 2
                    sl = 0 if b4 else s % 2
                    q_t, k_t, vp = qts[i], kts[i], vts[i]
                    sq0 = sl * N
                    if i not in dalls:
                        # one denominator tile per qkv tile (2 samples for
                        # b<3) so the reciprocal + R broadcast runs at full
                        # fp32r rate (free dim >= 256)
                        dalls[i] = rwpool.tile([128, CO, FQ], f32, tag="dall",
                                               bufs=2, name=f"d_{b}_{i}")
                        if FQ > FQv:
                            nc.gpsimd.memset(dalls[i][:, :, FQv:FQ], 1.0)
                    dall = dalls[i]
                    for h in range(H):
                        co, po = h // 2, (h % 2) * HD
                        ets = []
                        for j, (t0, kcp) in enumerate(kcs):
                            sp = mm_ps(kcp, nseq, f"s_{b}_{s}_{h}_{j}")
                            nc.tensor.matmul(
                                sp,
                                lhsT=k_t[po:po + HD, co,
                                         sq0 + t0:sq0 + t0 + kcp],
                                rhs=q_t[po:po + HD, co, sq0:sq0 + nseq],
                                start=True, stop=True)
                            e_t = epool.tile([128, nseq], bf16, tag="e",
                                             name=f"e_{b}_{s}_{h}_{j}")
                            nc.scalar.activation(
                                e_t[0:kcp], sp, AF.Exp,
                                bias=negm[0:kcp, s:s + 1, j], scale=SCL)
                            ets.append((e_t, kcp))
                        op = mm_ps(HD + 1, nseq, f"o_{b}_{s}_{h}")
                        for j, (e_t, kcp) in enumerate(ets):
                            nc.tensor.matmul(
                                op, lhsT=vp[0:kcp, sl, j, h, :],
                                rhs=e_t[0:kcp, :],
                                start=(j == 0), stop=(j == len(kcs) - 1))
                        nc.vector.tensor_copy(
                            atts[i][po:po + HD, co, sq0:sq0 + nseq]
                            .bitcast(f32r), op[0:HD, :])
                        p_d = (h % 2) * 64
                        nc.vector.tensor_copy(
                            dall[p_d:p_d + 1, h // 2, sq0:sq0 + nseq],
                            op[HD:HD + 1, :])
                    # masked-token columns stay unmasked garbage here: every
                    # consumer re-masks and the final LN masks the output.
                    if (not b4 and sl == 1) or b4:
                        rall = rwpool.tile([128, CO, FQ], f32, tag="dall",
                                           bufs=2, name=f"r_{b}_{i}")
                        with nc.allow_low_precision(
                                reason="fp32r rounding of softmax recip"):
                            nc.vector.reciprocal(rall[0:1].bitcast(f32r),
                                                 dall[0:1])
                            nc.vector.reciprocal(rall[64:65].bitcast(f32r),
                                                 dall[64:65])
                        for co in range(CO):
                            rps = mm_ps(128, FQ, f"R_{b}_{i}_{co}")
                            nc.tensor.matmul(rps, lhsT=e2[0:1, :],
                                             rhs=rall[0:1, co, :].bitcast(f32r),
                                             start=True, stop=False)
                            nc.tensor.matmul(
                                rps, lhsT=e2[64:65, :],
                                rhs=rall[64:65, co, :].bitcast(f32r),
                                start=False, stop=True)
                            nc.vector.tensor_tensor(
                                atts[i][:, co, :].bitcast(f32r),
                                atts[i][:, co, :], rps, OP.mult)

                # ---- proj + residual (per superblock) ----
                wp = wproj_d.ap()[b]
                wpt = [wtile(wp[c * 128:(c + 1) * 128, :], f"wp_{b}_{sb}_{c}")
                       for c in range(CO)]
                for i in iis:
                    xs = xspool.tile([128, CO, FQ], f32, tag="xst",
                                     name=f"xr_{b}_{i}")
                    nc.sync.dma_start(xs[:, :, 0:FQv], x_ap(i * FQv, FQv))
                    if FQ > FQv:
                        zero_fill_r(xs[:, :, FQv:FQ])
                    for o in range(CO):
                        ps = mm_ps(128, FQ, f"p_{b}_{i}_{o}")
                        for c in range(CO):
                            nc.tensor.matmul(
                                ps,
                                lhsT=wpt[c][:, o * 128:(o + 1) * 128]
                                    .bitcast(f32r),
                                rhs=atts[i][:, c, :].bitcast(f32r),
                                start=(c == 0), stop=(c == CO - 1))
                        nc.vector.tensor_tensor(xs[:, o, :], ps, xs[:, o, :],
                                                OP.add)
                    nc.sync.dma_start(x_ap(i * FQv, FQv), xs[:, :, 0:FQv])

            # ---- LN2 + MLP: superblocks of 516 tokens, pass-interleaved ----
            w1 = wfc1_d.ap()[b]
            w2 = wfc2_d.ap()[b]
            FM = 258
            for sb0 in range(0, TT, 516):
                xm2s, xss = [], []
                for tci in range(2):
                    t0 = sb0 + tci * FM
                    xm2 = emit_ln(
                        x_ap(t0, FM), mrow[0:1, t0:t0 + FM],
                        mbc[:, t0:t0 + FM] if b2 is not None else None,
                        g2, b2, FM, f"ln2_{b}_{sb0}_{tci}")
                    xm2s.append(xm2)
                    xs = xspool.tile([128, CO, FM], f32, tag="xst",
                                     name=f"xf_{b}_{sb0}_{tci}")
                    nc.sync.dma_start(xs, x_ap(t0, FM))
                    xss.append(xs)
                for p in range(4):
                    w1p = [wtile(w1[c * 128:(c + 1) * 128,
                                    p * C:(p + 1) * C],
                                 f"w1_{b}_{sb0}_{p}_{c}") for c in range(CO)]
                    hps = []
                    for tci in range(2):
                        hp = hppool.tile([128, CO, FM], f32, tag="hp",
                                         name=f"h_{b}_{sb0}_{p}_{tci}")
                        for o in range(CO):
                            ps = mm_ps(128, FM, f"f1_{b}_{sb0}_{p}_{tci}_{o}")
                            for c in range(CO):
                                nc.tensor.matmul(
                                    ps,
                                    lhsT=w1p[c][:, o * 128:(o + 1) * 128]
                                        .bitcast(f32r),
                                    rhs=xm2s[tci][:, c, :].bitcast(f32r),
                                    start=(c == 0), stop=(c == CO - 1))
                            nc.scalar.activation(hp[:, o, :].bitcast(f32r),
                                                 ps, AF.Gelu)
                        hps.append(hp)
                    w2p = [wtile(w2[p * C + j * 128:p * C + (j + 1) * 128, :],
                                 f"w2_{b}_{sb0}_{p}_{j}") for j in range(CO)]
                    for tci in range(2):
                        for o in range(CO):
                            ps = mm_ps(128, FM, f"f2_{b}_{sb0}_{p}_{tci}_{o}")
                            for j in range(CO):
                                nc.tensor.matmul(
                                    ps,
                                    lhsT=w2p[j][:, o * 128:(o + 1) * 128]
                                        .bitcast(f32r),
                                    rhs=hps[tci][:, j, :].bitcast(f32r),
                                    start=(j == 0), stop=(j == CO - 1))
                            nc.vector.tensor_tensor(
                                xss[tci][:, o, :], ps, xss[tci][:, o, :],
                                OP.add)
                for tci in range(2):
                    t0 = sb0 + tci * FM
                    if b4:
                        nc.sync.dma_start(x_ap(t0, FM), xss[tci])
                    else:
                        s0 = t0 // N
                        for sl in range(2):
                            dst = xd4s[:, :, s0 + sl,
                                       b * N:(b + 1) * N].rearrange(
                                           "co ci n -> ci co n")
                            nc.sync.dma_start(
                                dst, xss[tci][:, :, sl * N:(sl + 1) * N])

        # ---------------- input transposes ----------------
        for bi, nm in enumerate(("RGB", "NIR", "TIR")):
            xi = xin[nm].ap().rearrange("s n c -> (s n) c")
            for s in range(S):
                xf = xmpool.tile([128, CO, N], f32, tag="xm",
                                 name=f"xf_{nm}_{s}")
                for (t0, tpp) in ((0, 128), (128, 1)):
                    xt = xspool.tile([tpp, C], f32, tag="xst",
                                     name=f"xt_{nm}_{s}_{t0}")
                    nc.sync.dma_start(xt, xi[s * N + t0:s * N + t0 + tpp, :])
                    for cb in range(CO):
                        ps = mmps.tile([128, tpp], f32, tag="mm",
                                       name=f"tp_{nm}_{s}_{t0}_{cb}")
                        nc.tensor.transpose(ps, xt[:, cb * 128:(cb + 1) * 128],
                                            ident[0:tpp, 0:tpp])
                        nc.scalar.copy(xf[:, cb, t0:t0 + tpp], ps)
                nc.sync.dma_start(
                    xb_dram[bi][:, :, s * N:(s + 1) * N]
                    .rearrange("co ci t -> ci co t"), xf)

        for b in range(4):
            emit_branch(b)

        # ---------------- final LN + output ----------------
        gf = ln_cols(outg_d, None, "gf")
        bf = ln_cols(outb_d, None, "bf") if has_bf else None
        yflat = y_d.ap().rearrange("s n c -> (s n) c")
        for i in range(S):
            t0 = i * N4
            xmf = emit_ln(
                xd4[:, :, t0:t0 + N4].rearrange("co ci t -> ci co t"),
                m3_row[0:1, t0:t0 + 388],
                m3_bc[:, t0:t0 + 388] if bf is not None else None,
                gf, bf, 388, f"lnf_{i}", Fv=N4)
            for (p0, tpp) in ((0, 128), (128, 128), (256, 128), (384, 3)):
                ot = xspool.tile([128, C], f32, tag="xst", name=f"ot_{i}_{p0}")
                for cb in range(CO):
                    ps = mmps.tile([tpp, 128], f32, tag="mm",
                                   name=f"otp_{i}_{p0}_{cb}")
                    nc.tensor.matmul(ps, lhsT=xmf[:, cb, p0:p0 + tpp],
                                     rhs=ident, is_transpose=True,
                                     start=True, stop=True)
                    nc.scalar.copy(ot[0:tpp, cb * 128:(cb + 1) * 128], ps)
                nc.sync.dma_start(yflat[t0 + p0:t0 + p0 + tpp, :], ot[0:tpp, :])

    nc.compile()
    return nc


def _get_nc(has_b1, has_b2, has_bf):
    key = (has_b1, has_b2, has_bf)
    if key not in _CACHE:
        _CACHE[key] = _build(*key)
    return _CACHE[key]


def kernel(_trace=False, **inputs):
    from concourse.bass_utils import run_bass_kernel_spmd

    f = lambda a: np.ascontiguousarray(np.asarray(a), dtype=np.float32)
    has_b1 = bool(np.any(np.asarray(inputs["ln_b1"]) != 0))
    has_b2 = bool(np.any(np.asarray(inputs["ln_b2"]) != 0))
    has_bf = bool(np.any(np.asarray(inputs["out_b"]) != 0))
    nc = _get_nc(has_b1, has_b2, has_bf)

    shared = {k: f(inputs[k]) for k in
              ("ln_g1", "ln_b1", "ln_g2", "ln_b2", "w_qkv", "w_proj",
               "w_fc1", "w_fc2", "out_g", "out_b")}
    in_maps = []
    for i in range(NCORES):
        sl = slice(i * S, (i + 1) * S)
        m = dict(shared)
        m["RGB"] = f(inputs["RGB"][sl])
        m["NIR"] = f(inputs["NIR"][sl])
        m["TIR"] = f(inputs["TIR"][sl])
        m["mask"] = f(inputs["mask"][sl])
        in_maps.append(m)

    res = run_bass_kernel_spmd(nc, in_maps, core_ids=list(range(NCORES)),
                               trace=_trace)
    out = np.concatenate([res.results[i]["y"] for i in range(NCORES)], axis=0)
    if _trace:
        kernel._last_result = res
    return out.astype(np.float32)
